# revision 14
# baseline (speedup 1.0000x reference)
"""Cross-attention Bass/Tile kernel for Trainium2, sharded over 8 NeuronCores.

Problem (fixed shapes): B=2, T=2048, C=1024, H=16 heads, D=64.
    q = x_q @ Wq + bq;  kv = x_kv @ Wkv + bkv;  k, v = split(kv)
    y = softmax(q k^T / sqrt(D)) v;  out = y @ Wo + bo

Sharding: 8 cores = 2 (batch) x 4 (head groups of 4 heads, 256 channels).
Each core computes its head-group's projections + attention + a partial
output projection; the host sums the 4 partials per batch and adds the
exact v-bias/output-bias terms (att rows sum to 1 => out += bv@Wo + bo).

v2 design (all on-chip compute bf16, PSUM f32):
  - x_q^T / x_kv^T are transposed + bf16-cast on the HOST and passed as
    inputs, eliminating all on-device PE transposes of x.
  - Projections: Q^T,K^T = Wq/k^T x^T (lhsT = W natural), V = x_kv@Wv
    natural (lhsT = x^T chunk).
  - Attention per (qb of 512 q, hc head-pair): S^T[tk,q] (row-packed 2
    heads), exp on ACT ([128,1024] insts, scale=1/8), then AV flipped:
    y[q, d] with lhsT = e2 chunks, rhs = V (free=64: bf16 runs 1
    cycle/row at any free size -> half the cost of the y^T orientation).
    Denominators via free-1 matmuls against a ones vector.
  - Normalization: DVE reciprocal + broadcast multiply; y -> y^T via the
    XBAR dma-transpose engine (14ns/16x128 tile), not PE.
  - Output projection from y^T; PSUM->SBUF bf16 copy on GPSIMD (idle
    engine), DMA out as bf16; host converts/sums in f32.

PSUM map (8 banks): smega [128,2,1024] f32 (S slots, tk-parity) = 4,
ymega [128,2,4,2,64] f32 (y slots, pass-parity) = 2, dps [128,2,8]
(denominators) = 1, utilB [128,512] = 1.  Phase-A projections borrow
smega/ymega as scratch before the first attention pass; woven units
(Q-proj of qb+1, PO of qb-1, phase-A leftovers) rotate through utilB and
the idle ymega slot.  PSUM "start" zeroing is bank-granular, so each
shared bank keeps exactly one open accumulation group window at a time
(start on first slice, stop on last).
"""

import numpy as np

B = 2
T = 2048
C = 1024
H = 16
D = 64
NCORES = 8
TPG = 4  # tensor-parallel group size (head groups)
HL = H // TPG  # heads per core = 4
CL = HL * D  # local channels = 256
P = 128
KC = C // P  # 8 contraction chunks for the projections
NT = T // P  # 16 token chunks of 128
NQ = 4  # q blocks of 512
QW = T // NQ  # 512
DC = CL // P  # 2 chunks of d_local (head pairs)

_CACHE = {}


def _build(no_denom=False, no_weave=False, lag=2, e2_bufs=4, N_WARM=128):
    import concourse.tile as tile
    from concourse import bacc, mybir

    f32 = mybir.dt.float32
    bf16 = mybir.dt.bfloat16
    Exp = mybir.ActivationFunctionType.Exp

    nc = bacc.Bacc("TRN2", target_bir_lowering=False, debug=False)

    xqt_d = nc.dram_tensor("xqt", [C, T], bf16, kind="ExternalInput")
    xkvt_d = nc.dram_tensor("xkvt", [C, T], bf16, kind="ExternalInput")
    wq_d = nc.dram_tensor("wq", [C, CL], bf16, kind="ExternalInput")
    wk_d = nc.dram_tensor("wk", [C, CL], bf16, kind="ExternalInput")
    wv_d = nc.dram_tensor("wv", [C, CL], bf16, kind="ExternalInput")
    wo_d = nc.dram_tensor("wo", [CL, C], bf16, kind="ExternalInput")
    bq_d = nc.dram_tensor("bq", [CL], f32, kind="ExternalInput")
    bk_d = nc.dram_tensor("bk", [CL], f32, kind="ExternalInput")
    out_d = nc.dram_tensor("out", [T, C], bf16, kind="ExternalOutput")

    with tile.TileContext(nc) as tc:
        with (
            tc.tile_pool(name="const", bufs=1) as const,
            tc.tile_pool(name="persist", bufs=1) as persist,
            tc.tile_pool(name="e2p", bufs=e2_bufs) as e2p,
            tc.tile_pool(name="small", bufs=2) as small,
            tc.tile_pool(name="ostage", bufs=4) as ostage,
        ):
            from concourse.masks import make_identity

            ones_bf = const.tile([P, 1], bf16)
            nc.vector.memset(ones_bf, 1.0)
            ident_f32 = const.tile([P, P], f32)
            make_identity(nc, ident_f32)
            ident_bf = const.tile([P, P], bf16)
            nc.vector.tensor_copy(ident_bf, ident_f32)

            wq_sb = const.tile([P, KC, CL], bf16)
            wk_sb = const.tile([P, KC, CL], bf16)
            wv_sb = const.tile([P, KC, CL], bf16)
            wo_sb = const.tile([P, DC, C], bf16)
            bq_sb = const.tile([P, DC], f32)
            bk_sb = const.tile([P, DC], f32)

            xqt_sb = persist.tile([P, KC, T], bf16)
            xkvt_sb = persist.tile([P, KC, T], bf16)
            qt_sb = persist.tile([P, DC, T], bf16)  # Q^T [d, t]
            kt_sb = persist.tile([P, DC, T], bf16)  # K^T [d, t]
            v_sb = persist.tile([P, NT, HL, D], bf16)  # V [tk, h, d]
            yt_sb = persist.tile([P, DC, T], bf16)  # y^T [d, t] normalized

            ps = tc.alloc_tile_pool(name="ps", bufs=1, space="PSUM")
            smega_t = [
                ps.tile([P, 2 * QW], f32, tag=f"smega{i}", name=f"smega{i}")
                for i in range(2)
            ]
            ymega_t = [
                ps.tile([P, NQ, 2, D], f32, tag=f"ymega{i}", name=f"ymega{i}")
                for i in range(2)
            ]
            dps = ps.tile([P, 2, 8], f32, tag="dps", name="dps")
            dps_t = [dps[:, 0], dps[:, 1]]
            utilB = ps.tile([P, QW], f32, tag="utilB", name="utilB")

            # ---- DMA emission: SWDGE (Pool) carries wk/wv/xkvt, HWDGE
            # (SP) carries wq/xqt/biases/wo.  Order = priority: the
            # phase-A prefix needs wk+xkvt(tb0) and wq+xqt(qb0) first.
            wk_src = wk_d.rearrange("(o p) d -> p o d", p=P)
            wv_src = wv_d.rearrange("(o p) d -> p o d", p=P)
            wq_src = wq_d.rearrange("(o p) d -> p o d", p=P)
            xkvt_src = xkvt_d.rearrange("(o p) t -> p o t", p=P)
            xqt_src = xqt_d.rearrange("(o p) t -> p o t", p=P)

            def xsl(tb):
                return slice(tb * QW, (tb + 1) * QW)

            nc.sync.dma_start(bk_sb, bk_d.rearrange("(o p) -> p o", p=P))
            nc.sync.dma_start(bq_sb, bq_d.rearrange("(o p) -> p o", p=P))
            nc.sync.dma_start(wk_sb, wk_src)
            nc.sync.dma_start(xkvt_sb[:, :, xsl(0)], xkvt_src[:, :, xsl(0)])
            nc.sync.dma_start(wq_sb, wq_src)
            nc.sync.dma_start(xqt_sb[:, :, xsl(0)], xqt_src[:, :, xsl(0)])
            nc.sync.dma_start(wv_sb, wv_src)
            for tb in range(1, NQ):
                nc.sync.dma_start(xkvt_sb[:, :, xsl(tb)], xkvt_src[:, :, xsl(tb)])
            nc.sync.dma_start(xqt_sb[:, :, xsl(1)], xqt_src[:, :, xsl(1)])
            nc.sync.dma_start(wo_sb, wo_d.rearrange("(o p) n -> p o n", p=P))
            nc.sync.dma_start(xqt_sb[:, :, xsl(2)], xqt_src[:, :, xsl(2)])
            nc.sync.dma_start(xqt_sb[:, :, xsl(3)], xqt_src[:, :, xsl(3)])

            # ---------- work units (each takes a [P, QW] f32 PSUM view) ----
            def u_proj(kind, idx, dc):
                x_sb, w_sb, b_sb, dst = (
                    (xqt_sb, wq_sb, bq_sb, qt_sb)
                    if kind == "q"
                    else (xkvt_sb, wk_sb, bk_sb, kt_sb)
                )

                def run(psv):
                    for c in range(KC):
                        nc.tensor.matmul(
                            psv,
                            w_sb[:, c, dc * P : (dc + 1) * P],
                            x_sb[:, c, idx * QW : (idx + 1) * QW],
                            start=(c == 0),
                            stop=(c == KC - 1),
                        )
                    nc.vector.tensor_scalar_add(
                        dst[:, dc, idx * QW : (idx + 1) * QW],
                        psv,
                        b_sb[:, dc : dc + 1],
                    )

                return run, 2.4

            def u_v(tkc):
                def run(psv):
                    v256 = psv[:, 0:CL]
                    for c in range(KC):
                        nc.tensor.matmul(
                            v256,
                            xkvt_sb[:, c, tkc * P : (tkc + 1) * P],
                            wv_sb[:, c, :],
                            start=(c == 0),
                            stop=(c == KC - 1),
                        )
                    nc.vector.tensor_copy(
                        v_sb[:, tkc], v256.rearrange("p (h d) -> p h d", h=HL)
                    )

                return run, 0.9

            def u_po(tch, half, eng=None):
                def run(psv):
                    for dc in range(DC):
                        nc.tensor.matmul(
                            psv,
                            yt_sb[:, dc, tch * P : (tch + 1) * P],
                            wo_sb[:, dc, half * QW : (half + 1) * QW],
                            start=(dc == 0),
                            stop=(dc == DC - 1),
                        )
                    o_st = ostage.tile([P, QW], bf16, tag="o", name="o_st")
                    (eng or nc.gpsimd).tensor_copy(o_st, psv)
                    nc.sync.dma_start(
                        out_d[tch * P : (tch + 1) * P, half * QW : (half + 1) * QW],
                        o_st,
                    )

                return run, 0.6

            # ---- PE warmup: harmless transposes during the DMA lead-in
            # keep the p-state ramp running so phase A runs at full clock
            for i in range(N_WARM):
                wps = utilB[:, 256 + (i % 4) * 64 : 320 + (i % 4) * 64].bitcast(bf16)
                nc.tensor.transpose(wps, ident_bf, ident_bf)

            # ---- phase A prefix: K(tb0,dc0) and Q(qb0,dc0) on smega
            # quarters (S(tk0) overwrites them later; subtile deps order it)
            kq_run, _ = u_proj("k", 0, 0)
            kq_run(smega_t[0][:, 0:QW])
            qq_run, _ = u_proj("q", 0, 0)
            qq_run(smega_t[0][:, QW : 2 * QW])

            # ---- attention passes ----
            def av_emit(tk, e2, hc, ydst, ddst):
                for qc in range(NQ):
                    for hh in range(2):
                        lhs = e2[:, hh * QW + qc * P : hh * QW + (qc + 1) * P]
                        nc.tensor.matmul(
                            ydst[:, qc, hh],
                            lhs,
                            v_sb[:, tk, hc * 2 + hh],
                            start=(tk == 0 and qc == 0 and hh == 0),
                            stop=(tk == NT - 1 and qc == NQ - 1 and hh == 1),
                        )
                if no_denom:
                    return
                for qc in range(NQ):
                    for hh in range(2):
                        s = qc * 2 + hh
                        lhs = e2[:, hh * QW + qc * P : hh * QW + (qc + 1) * P]
                        nc.tensor.matmul(
                            ddst[:, s : s + 1],
                            lhs,
                            ones_bf,
                            start=(tk == 0 and s == 0),
                            stop=(tk == NT - 1 and s == 7),
                        )

            def emit_pass(qb, hc, pass_idx, weave, prev_tail=None, po_after_qc=None):
                hcpar = pass_idx % 2
                other = 1 - hcpar
                ydst = ymega_t[hcpar]  # [P, NQ, 2, D]
                ddst = dps_t[hcpar]  # [P, 8]
                homes = [
                    utilB,
                    ymega_t[other].rearrange("p a b d -> p (a b d)"),
                ]
                home_i = [0]

                def pop_units(budget):
                    while weave and budget > 0.0:
                        run, cost = weave.pop(0)
                        run(homes[home_i[0] % 2])
                        home_i[0] += 1
                        budget -= cost

                pend = []
                for tk in range(NT):
                    par = tk % 2
                    for hh in range(2):
                        nc.tensor.matmul(
                            smega_t[par][:, hh * QW : (hh + 1) * QW],
                            kt_sb[hh * 64 : (hh + 1) * 64, hc, tk * P : (tk + 1) * P],
                            qt_sb[hh * 64 : (hh + 1) * 64, hc, qb * QW : (qb + 1) * QW],
                            start=True,
                            stop=True,
                            tile_position=(hh * 64, 0),
                        )
                    e2 = e2p.tile([P, 2 * QW], bf16, tag="e2", name="e2")
                    nc.scalar.activation(e2, smega_t[par], Exp, scale=0.125)
                    pend.append((tk, e2))
                    if len(pend) > lag:
                        ptk, pe2 = pend.pop(0)
                        av_emit(ptk, pe2, hc, ydst, ddst)
                    if tk == 2 and prev_tail is not None:
                        prev_tail()
                    pop_units(1.6)

                def tail():
                    while pend:
                        ptk, pe2 = pend.pop(0)
                        av_emit(ptk, pe2, hc, ydst, ddst)
                    while weave:
                        run, _ = weave.pop(0)
                        run(homes[home_i[0] % 2])
                        home_i[0] += 1
                    # normalize + PE-transpose to y^T
                    recip = small.tile([P, 8], f32, tag="recip", name="recip")
                    dsrc = (
                        ddst
                        if not no_denom
                        else ymega_t[hcpar].rearrange("p a b d -> p (a b d)")[:, 0:8]
                    )
                    nc.vector.reciprocal(recip, dsrc)
                    yn = small.tile([P, NQ, 2, D], bf16, tag="yn", name="yn")
                    try:
                        rb = (
                            recip.rearrange("p (a b) -> p a b", a=NQ)
                            .unsqueeze(-1)
                            .broadcast_to((P, NQ, 2, D))
                        )
                        nc.vector.tensor_mul(out=yn, in0=ydst, in1=rb)
                    except Exception:
                        for s in range(8):
                            qc, hh = s // 2, s % 2
                            nc.vector.tensor_scalar_mul(
                                yn[:, qc, hh], ydst[:, qc, hh], recip[:, s : s + 1]
                            )
                    for qc in range(NQ):
                        tp_ps = utilB[:, qc * 64 : (qc + 1) * 64].bitcast(bf16)
                        nc.tensor.transpose(
                            tp_ps,
                            yn[:, qc].rearrange("p a d -> p (a d)"),
                            ident_bf,
                        )
                        nc.vector.tensor_copy(
                            yt_sb[:, hc, qb * QW + qc * P : qb * QW + (qc + 1) * P],
                            tp_ps,
                        )
                        if po_after_qc is not None:
                            for run, home in po_after_qc(qc):
                                run(home)

                return tail

            # weave lists per pass (see docstring): phase-A leftovers into
            # pass 0/1, Q-proj of qb+1 into (qb, hc1), PO of qb-1 into
            # (qb, hc0)/(qb, hc1).
            weaves = [[] for _ in range(8)]
            weaves[0] = [
                u_v(0), u_v(1), u_v(2), u_v(3),
                u_proj("k", 1, 0),
                u_v(4), u_v(5), u_v(6),
                u_proj("k", 2, 0),
                u_v(7), u_v(8),
                u_proj("k", 3, 0),
                u_v(9), u_v(10), u_v(11),
                u_proj("k", 0, 1),
                u_v(12), u_v(13), u_v(14), u_v(15),
            ]
            weaves[1] = [
                u_proj("k", 1, 1),
                u_proj("q", 1, 0),
                u_proj("k", 2, 1),
                u_proj("q", 1, 1),
                u_proj("k", 3, 1),
            ]
            weaves[2] = [u_po(0 * 4 + i, h) for i in range(4) for h in range(2)]
            weaves[3] = [u_proj("q", 2, 0), u_proj("q", 2, 1)]
            weaves[4] = [u_po(1 * 4 + i, h) for i in range(4) for h in range(2)]
            weaves[5] = [u_proj("q", 3, 0), u_proj("q", 3, 1)]
            weaves[6] = [u_po(2 * 4 + i, h) for i in range(4) for h in range(2)]
            weaves[7] = []

            if no_weave:
                homesA = [
                    smega_t[i // 2][:, (i % 2) * QW : (i % 2 + 1) * QW]
                    for i in range(4)
                ] + [
                    ymega_t[0].rearrange("p a b d -> p (a b d)"),
                    ymega_t[1].rearrange("p a b d -> p (a b d)"),
                    utilB,
                ]
                k = 0
                for w in weaves:
                    while w:
                        run, _ = w.pop(0)
                        run(homesA[k % 7])
                        k += 1
            tail_homes = [
                smega_t[0][:, 0:QW],
                smega_t[0][:, QW : 2 * QW],
                smega_t[1][:, 0:QW],
                smega_t[1][:, QW : 2 * QW],
                ymega_t[0].rearrange("p a b d -> p (a b d)"),
                ymega_t[1].rearrange("p a b d -> p (a b d)"),
                smega_t[0][:, 0:QW],
                smega_t[0][:, QW : 2 * QW],
            ]

            def last_po(qc):
                tch = 3 * 4 + qc

                def run_pair(_ignored):
                    o2 = ostage.tile([P, 2 * QW], bf16, tag="o2", name="o2", bufs=2)
                    for h in range(2):
                        psv = tail_homes[qc * 2 + h]
                        for dc in range(DC):
                            nc.tensor.matmul(
                                psv,
                                yt_sb[:, dc, tch * P : (tch + 1) * P],
                                wo_sb[:, dc, h * QW : (h + 1) * QW],
                                start=(dc == 0),
                                stop=(dc == DC - 1),
                            )
                        eng = nc.vector if h == 0 else nc.gpsimd
                        eng.tensor_copy(o2[:, h * QW : (h + 1) * QW], psv)
                    nc.sync.dma_start(out_d[tch * P : (tch + 1) * P, :], o2)

                return [(run_pair, None)]

            prev_tail = None
            pass_idx = 0
            for qb in range(NQ):
                for hc in range(DC):
                    prev_tail = emit_pass(
                        qb,
                        hc,
                        pass_idx,
                        weaves[pass_idx],
                        prev_tail,
                        po_after_qc=(last_po if pass_idx == 7 else None),
                    )
                    pass_idx += 1
            prev_tail()

            ps.release()

    nc.compile()
    return nc


def _get_nc():
    if "nc" not in _CACHE:
        _CACHE["nc"] = _build()
    return _CACHE["nc"]


def _shard_inputs(x_q, x_kv, Wq, bq, Wkv, bkv, Wo=None, bo=None):
    import ml_dtypes

    bf = ml_dtypes.bfloat16
    in_maps = []
    for core in range(NCORES):
        b = core // TPG
        g = core % TPG
        cols = slice(g * CL, (g + 1) * CL)
        m = {
            "xqt": np.ascontiguousarray(x_q[b].T.astype(bf)),
            "xkvt": np.ascontiguousarray(x_kv[b].T.astype(bf)),
            "wq": np.ascontiguousarray(Wq[:, cols].astype(bf)),
            "wk": np.ascontiguousarray(Wkv[:, :C][:, cols].astype(bf)),
            "wv": np.ascontiguousarray(Wkv[:, C:][:, cols].astype(bf)),
            "bq": np.ascontiguousarray(bq[cols].astype(np.float32)),
            "bk": np.ascontiguousarray(bkv[:C][cols].astype(np.float32)),
        }
        if Wo is not None:
            m["wo"] = np.ascontiguousarray(Wo[g * CL : (g + 1) * CL, :].astype(bf))
        in_maps.append(m)
    return in_maps


def kernel(x_q, x_kv, Wq, bq, Wkv, bkv, Wo, bo):
    from concourse.bass_utils import run_bass_kernel_spmd

    x_q = np.asarray(x_q, dtype=np.float32)
    x_kv = np.asarray(x_kv, dtype=np.float32)
    Wq = np.asarray(Wq, dtype=np.float32)
    bq = np.asarray(bq, dtype=np.float32)
    Wkv = np.asarray(Wkv, dtype=np.float32)
    bkv = np.asarray(bkv, dtype=np.float32)
    Wo = np.asarray(Wo, dtype=np.float32)
    bo = np.asarray(bo, dtype=np.float32)

    nc = _get_nc()
    in_maps = _shard_inputs(x_q, x_kv, Wq, bq, Wkv, bkv, Wo, bo)

    res = run_bass_kernel_spmd(nc, in_maps, core_ids=list(range(NCORES)))

    # host-side gather: sum tensor-parallel partials; add exact bias terms
    bias_full = bkv[C:] @ Wo + bo  # v-bias through Wo, plus output bias
    out = np.zeros((B, T, C), dtype=np.float32)
    for core in range(NCORES):
        out[core // TPG] += np.asarray(res.results[core]["out"], dtype=np.float32)
    out += bias_full[None, None, :]
    return out


# revision 15
# speedup vs baseline: 1.0090x; 1.0090x over previous
"""Cross-attention Bass/Tile kernel for Trainium2, sharded over 8 NeuronCores.

Problem (fixed shapes): B=2, T=2048, C=1024, H=16 heads, D=64.
    q = x_q @ Wq + bq;  kv = x_kv @ Wkv + bkv;  k, v = split(kv)
    y = softmax(q k^T / sqrt(D)) v;  out = y @ Wo + bo

Sharding: 8 cores = 2 (batch) x 4 (head groups of 4 heads, 256 channels).
Each core computes its head-group's projections + attention + a partial
output projection; the host sums the 4 partials per batch and adds the
exact v-bias/output-bias terms (att rows sum to 1 => out += bv@Wo + bo).

v2 design (all on-chip compute bf16, PSUM f32):
  - x_q^T / x_kv^T are transposed + bf16-cast on the HOST and passed as
    inputs, eliminating all on-device PE transposes of x.
  - Projections: Q^T,K^T = Wq/k^T x^T (lhsT = W natural), V = x_kv@Wv
    natural (lhsT = x^T chunk).
  - Attention per (qb of 512 q, hc head-pair): S^T[tk,q] (row-packed 2
    heads), exp on ACT ([128,1024] insts, scale=1/8), then AV flipped:
    y[q, d] with lhsT = e2 chunks, rhs = V (free=64: bf16 runs 1
    cycle/row at any free size -> half the cost of the y^T orientation).
    Denominators via free-1 matmuls against a ones vector.
  - Normalization: DVE reciprocal + broadcast multiply; y -> y^T via the
    XBAR dma-transpose engine (14ns/16x128 tile), not PE.
  - Output projection from y^T; PSUM->SBUF bf16 copy on GPSIMD (idle
    engine), DMA out as bf16; host converts/sums in f32.

PSUM map (8 banks): smega [128,2,1024] f32 (S slots, tk-parity) = 4,
ymega [128,2,4,2,64] f32 (y slots, pass-parity) = 2, dps [128,2,8]
(denominators) = 1, utilB [128,512] = 1.  Phase-A projections borrow
smega/ymega as scratch before the first attention pass; woven units
(Q-proj of qb+1, PO of qb-1, phase-A leftovers) rotate through utilB and
the idle ymega slot.  PSUM "start" zeroing is bank-granular, so each
shared bank keeps exactly one open accumulation group window at a time
(start on first slice, stop on last).
"""

import numpy as np

B = 2
T = 2048
C = 1024
H = 16
D = 64
NCORES = 8
TPG = 4  # tensor-parallel group size (head groups)
HL = H // TPG  # heads per core = 4
CL = HL * D  # local channels = 256
P = 128
KC = C // P  # 8 contraction chunks for the projections
NT = T // P  # 16 token chunks of 128
NQ = 4  # q blocks of 512
QW = T // NQ  # 512
DC = CL // P  # 2 chunks of d_local (head pairs)

_CACHE = {}


def _build(no_denom=False, no_weave=False, lag=2, e2_bufs=4, N_WARM=128):
    import concourse.tile as tile
    from concourse import bacc, mybir

    f32 = mybir.dt.float32
    bf16 = mybir.dt.bfloat16
    Exp = mybir.ActivationFunctionType.Exp

    nc = bacc.Bacc("TRN2", target_bir_lowering=False, debug=False)

    xqt_d = nc.dram_tensor("xqt", [C, T], bf16, kind="ExternalInput")
    xkvt_d = nc.dram_tensor("xkvt", [C, T], bf16, kind="ExternalInput")
    wq_d = nc.dram_tensor("wq", [C, CL], bf16, kind="ExternalInput")
    wk_d = nc.dram_tensor("wk", [C, CL], bf16, kind="ExternalInput")
    wv_d = nc.dram_tensor("wv", [C, CL], bf16, kind="ExternalInput")
    wo_d = nc.dram_tensor("wo", [CL, C], bf16, kind="ExternalInput")
    bq_d = nc.dram_tensor("bq", [CL], f32, kind="ExternalInput")
    bk_d = nc.dram_tensor("bk", [CL], f32, kind="ExternalInput")
    out_d = nc.dram_tensor("out", [T, C], bf16, kind="ExternalOutput")

    with tile.TileContext(nc) as tc:
        with (
            tc.tile_pool(name="const", bufs=1) as const,
            tc.tile_pool(name="persist", bufs=1) as persist,
            tc.tile_pool(name="e2p", bufs=e2_bufs) as e2p,
            tc.tile_pool(name="small", bufs=2) as small,
            tc.tile_pool(name="ostage", bufs=4) as ostage,
        ):
            from concourse.masks import make_identity

            ones_bf = const.tile([P, 1], bf16)
            nc.vector.memset(ones_bf, 1.0)
            ident_f32 = const.tile([P, P], f32)
            make_identity(nc, ident_f32)
            ident_bf = const.tile([P, P], bf16)
            nc.vector.tensor_copy(ident_bf, ident_f32)

            wq_sb = const.tile([P, KC, CL], bf16)
            wk_sb = const.tile([P, KC, CL], bf16)
            wv_sb = const.tile([P, KC, CL], bf16)
            wo_sb = const.tile([P, DC, C], bf16)
            bq_sb = const.tile([P, DC], f32)
            bk_sb = const.tile([P, DC], f32)

            xqt_sb = persist.tile([P, KC, T], bf16)
            xkvt_sb = persist.tile([P, KC, T], bf16)
            qt_sb = persist.tile([P, DC, T], bf16)  # Q^T [d, t]
            kt_sb = persist.tile([P, DC, T], bf16)  # K^T [d, t]
            v_sb = persist.tile([P, NT, HL, D], bf16)  # V [tk, h, d]
            yt_sb = persist.tile([P, DC, T], bf16)  # y^T [d, t] normalized

            ps = tc.alloc_tile_pool(name="ps", bufs=1, space="PSUM")
            smega_t = [
                ps.tile([P, 2 * QW], f32, tag=f"smega{i}", name=f"smega{i}")
                for i in range(2)
            ]
            ymega_t = [
                ps.tile([P, NQ, 2, D], f32, tag=f"ymega{i}", name=f"ymega{i}")
                for i in range(2)
            ]
            dps = ps.tile([P, 2, 8], f32, tag="dps", name="dps")
            dps_t = [dps[:, 0], dps[:, 1]]
            utilB = ps.tile([P, QW], f32, tag="utilB", name="utilB")

            # ---- DMA emission: SWDGE (Pool) carries wk/wv/xkvt, HWDGE
            # (SP) carries wq/xqt/biases/wo.  Order = priority: the
            # phase-A prefix needs wk+xkvt(tb0) and wq+xqt(qb0) first.
            wk_src = wk_d.rearrange("(o p) d -> p o d", p=P)
            wv_src = wv_d.rearrange("(o p) d -> p o d", p=P)
            wq_src = wq_d.rearrange("(o p) d -> p o d", p=P)
            xkvt_src = xkvt_d.rearrange("(o p) t -> p o t", p=P)
            xqt_src = xqt_d.rearrange("(o p) t -> p o t", p=P)

            def xsl(tb):
                return slice(tb * QW, (tb + 1) * QW)

            nc.sync.dma_start(bk_sb, bk_d.rearrange("(o p) -> p o", p=P))
            nc.sync.dma_start(bq_sb, bq_d.rearrange("(o p) -> p o", p=P))
            nc.sync.dma_start(wk_sb, wk_src)
            nc.sync.dma_start(xkvt_sb[:, :, xsl(0)], xkvt_src[:, :, xsl(0)])
            nc.sync.dma_start(wq_sb, wq_src)
            nc.sync.dma_start(xqt_sb[:, :, xsl(0)], xqt_src[:, :, xsl(0)])
            nc.sync.dma_start(wv_sb, wv_src)
            for tb in range(1, NQ):
                nc.sync.dma_start(xkvt_sb[:, :, xsl(tb)], xkvt_src[:, :, xsl(tb)])
            nc.sync.dma_start(xqt_sb[:, :, xsl(1)], xqt_src[:, :, xsl(1)])
            nc.sync.dma_start(wo_sb, wo_d.rearrange("(o p) n -> p o n", p=P))
            nc.sync.dma_start(xqt_sb[:, :, xsl(2)], xqt_src[:, :, xsl(2)])
            nc.sync.dma_start(xqt_sb[:, :, xsl(3)], xqt_src[:, :, xsl(3)])

            # ---------- work units (each takes a [P, QW] f32 PSUM view) ----
            def u_proj(kind, idx, dc):
                x_sb, w_sb, b_sb, dst = (
                    (xqt_sb, wq_sb, bq_sb, qt_sb)
                    if kind == "q"
                    else (xkvt_sb, wk_sb, bk_sb, kt_sb)
                )

                def run(psv):
                    for c in range(KC):
                        nc.tensor.matmul(
                            psv,
                            w_sb[:, c, dc * P : (dc + 1) * P],
                            x_sb[:, c, idx * QW : (idx + 1) * QW],
                            start=(c == 0),
                            stop=(c == KC - 1),
                        )
                    nc.vector.tensor_scalar_add(
                        dst[:, dc, idx * QW : (idx + 1) * QW],
                        psv,
                        b_sb[:, dc : dc + 1],
                    )

                return run, 2.4

            def u_v(tkc):
                def run(psv):
                    v256 = psv[:, 0:CL]
                    for c in range(KC):
                        nc.tensor.matmul(
                            v256,
                            xkvt_sb[:, c, tkc * P : (tkc + 1) * P],
                            wv_sb[:, c, :],
                            start=(c == 0),
                            stop=(c == KC - 1),
                        )
                    nc.vector.tensor_copy(
                        v_sb[:, tkc], v256.rearrange("p (h d) -> p h d", h=HL)
                    )

                return run, 0.9

            def u_po(tch, half, eng=None):
                def run(psv):
                    for dc in range(DC):
                        nc.tensor.matmul(
                            psv,
                            yt_sb[:, dc, tch * P : (tch + 1) * P],
                            wo_sb[:, dc, half * QW : (half + 1) * QW],
                            start=(dc == 0),
                            stop=(dc == DC - 1),
                        )
                    o_st = ostage.tile([P, QW], bf16, tag="o", name="o_st")
                    (eng or nc.gpsimd).tensor_copy(o_st, psv)
                    nc.sync.dma_start(
                        out_d[tch * P : (tch + 1) * P, half * QW : (half + 1) * QW],
                        o_st,
                    )

                return run, 0.6

            # ---- PE warmup: harmless transposes during the DMA lead-in
            # keep the p-state ramp running so phase A runs at full clock
            for i in range(N_WARM):
                wps = utilB[:, 256 + (i % 4) * 64 : 320 + (i % 4) * 64].bitcast(bf16)
                nc.tensor.transpose(wps, ident_bf, ident_bf)

            # ---- phase A prefix: K(tb0,dc0) and Q(qb0,dc0) on smega
            # quarters (S(tk0) overwrites them later; subtile deps order it)
            kq_run, _ = u_proj("k", 0, 0)
            kq_run(smega_t[0][:, 0:QW])
            qq_run, _ = u_proj("q", 0, 0)
            qq_run(smega_t[0][:, QW : 2 * QW])

            # ---- attention passes ----
            def av_emit(tk, e2, hc, ydst, ddst):
                for qc in range(NQ):
                    for hh in range(2):
                        lhs = e2[:, hh * QW + qc * P : hh * QW + (qc + 1) * P]
                        nc.tensor.matmul(
                            ydst[:, qc, hh],
                            lhs,
                            v_sb[:, tk, hc * 2 + hh],
                            start=(tk == 0 and qc == 0 and hh == 0),
                            stop=(tk == NT - 1 and qc == NQ - 1 and hh == 1),
                        )
                if no_denom:
                    return
                for qc in range(NQ):
                    for hh in range(2):
                        s = qc * 2 + hh
                        lhs = e2[:, hh * QW + qc * P : hh * QW + (qc + 1) * P]
                        nc.tensor.matmul(
                            ddst[:, s : s + 1],
                            lhs,
                            ones_bf,
                            start=(tk == 0 and s == 0),
                            stop=(tk == NT - 1 and s == 7),
                        )

            def emit_pass(qb, hc, pass_idx, weave, prev_tail=None, po_after_qc=None):
                hcpar = pass_idx % 2
                other = 1 - hcpar
                ydst = ymega_t[hcpar]  # [P, NQ, 2, D]
                ddst = dps_t[hcpar]  # [P, 8]
                homes = [
                    utilB,
                    ymega_t[other].rearrange("p a b d -> p (a b d)"),
                ]
                home_i = [0]

                def pop_units(budget):
                    while weave and budget > 0.0:
                        run, cost = weave.pop(0)
                        run(homes[home_i[0] % 2])
                        home_i[0] += 1
                        budget -= cost

                pend = []
                for tk in range(NT):
                    par = tk % 2
                    for hh in range(2):
                        nc.tensor.matmul(
                            smega_t[par][:, hh * QW : (hh + 1) * QW],
                            kt_sb[hh * 64 : (hh + 1) * 64, hc, tk * P : (tk + 1) * P],
                            qt_sb[hh * 64 : (hh + 1) * 64, hc, qb * QW : (qb + 1) * QW],
                            start=True,
                            stop=True,
                            tile_position=(hh * 64, 0),
                        )
                    e2 = e2p.tile([P, 2 * QW], bf16, tag="e2", name="e2")
                    nc.scalar.activation(e2, smega_t[par], Exp, scale=0.125)
                    pend.append((tk, e2))
                    if len(pend) > lag:
                        ptk, pe2 = pend.pop(0)
                        av_emit(ptk, pe2, hc, ydst, ddst)
                    if tk == 2 and prev_tail is not None:
                        prev_tail()
                    pop_units(1.6)

                def tail():
                    while pend:
                        ptk, pe2 = pend.pop(0)
                        av_emit(ptk, pe2, hc, ydst, ddst)
                    while weave:
                        run, _ = weave.pop(0)
                        run(homes[home_i[0] % 2])
                        home_i[0] += 1
                    # normalize + PE-transpose to y^T
                    recip = small.tile([P, 8], f32, tag="recip", name="recip")
                    dsrc = (
                        ddst
                        if not no_denom
                        else ymega_t[hcpar].rearrange("p a b d -> p (a b d)")[:, 0:8]
                    )
                    nc.vector.reciprocal(recip, dsrc)
                    yn = small.tile([P, NQ, 2, D], bf16, tag="yn", name="yn")
                    try:
                        rb = (
                            recip.rearrange("p (a b) -> p a b", a=NQ)
                            .unsqueeze(-1)
                            .broadcast_to((P, NQ, 2, D))
                        )
                        nc.vector.tensor_mul(out=yn, in0=ydst, in1=rb)
                    except Exception:
                        for s in range(8):
                            qc, hh = s // 2, s % 2
                            nc.vector.tensor_scalar_mul(
                                yn[:, qc, hh], ydst[:, qc, hh], recip[:, s : s + 1]
                            )
                    for qc in range(NQ):
                        tp_ps = utilB[:, qc * 64 : (qc + 1) * 64].bitcast(bf16)
                        nc.tensor.transpose(
                            tp_ps,
                            yn[:, qc].rearrange("p a d -> p (a d)"),
                            ident_bf,
                        )
                        nc.vector.tensor_copy(
                            yt_sb[:, hc, qb * QW + qc * P : qb * QW + (qc + 1) * P],
                            tp_ps,
                        )
                        if po_after_qc is not None:
                            for run, home in po_after_qc(qc):
                                run(home)

                return tail

            # weave lists per pass (see docstring): phase-A leftovers into
            # pass 0/1, Q-proj of qb+1 into (qb, hc1), PO of qb-1 into
            # (qb, hc0)/(qb, hc1).
            weaves = [[] for _ in range(8)]
            weaves[0] = [
                u_v(0), u_v(1), u_v(2), u_v(3),
                u_proj("k", 1, 0),
                u_v(4), u_v(5), u_v(6),
                u_proj("k", 2, 0),
                u_v(7), u_v(8),
                u_proj("k", 3, 0),
                u_v(9), u_v(10), u_v(11),
                u_proj("k", 0, 1),
                u_v(12), u_v(13), u_v(14), u_v(15),
            ]
            weaves[1] = [
                u_proj("k", 1, 1),
                u_proj("q", 1, 0),
                u_proj("k", 2, 1),
                u_proj("q", 1, 1),
                u_proj("k", 3, 1),
            ]
            weaves[2] = [u_po(0 * 4 + i, h) for i in range(4) for h in range(2)]
            weaves[3] = [u_proj("q", 2, 0), u_proj("q", 2, 1)]
            weaves[4] = [u_po(1 * 4 + i, h) for i in range(4) for h in range(2)]
            weaves[5] = [u_proj("q", 3, 0), u_proj("q", 3, 1)]
            weaves[6] = [u_po(2 * 4 + i, h) for i in range(4) for h in range(2)]
            weaves[7] = []

            if no_weave:
                homesA = [
                    smega_t[i // 2][:, (i % 2) * QW : (i % 2 + 1) * QW]
                    for i in range(4)
                ] + [
                    ymega_t[0].rearrange("p a b d -> p (a b d)"),
                    ymega_t[1].rearrange("p a b d -> p (a b d)"),
                    utilB,
                ]
                k = 0
                for w in weaves:
                    while w:
                        run, _ = w.pop(0)
                        run(homesA[k % 7])
                        k += 1
            tail_homes = [
                smega_t[0][:, 0:QW],
                smega_t[0][:, QW : 2 * QW],
                smega_t[1][:, 0:QW],
                smega_t[1][:, QW : 2 * QW],
                ymega_t[0].rearrange("p a b d -> p (a b d)"),
                ymega_t[1].rearrange("p a b d -> p (a b d)"),
                smega_t[0][:, 0:QW],
                smega_t[0][:, QW : 2 * QW],
            ]

            def last_po(qc):
                return [
                    (
                        u_po(3 * 4 + qc, h, eng=(nc.vector if h == 0 else nc.gpsimd))[0],
                        tail_homes[qc * 2 + h],
                    )
                    for h in range(2)
                ]

            prev_tail = None
            pass_idx = 0
            for qb in range(NQ):
                for hc in range(DC):
                    prev_tail = emit_pass(
                        qb,
                        hc,
                        pass_idx,
                        weaves[pass_idx],
                        prev_tail,
                        po_after_qc=(last_po if pass_idx == 7 else None),
                    )
                    pass_idx += 1
            prev_tail()

            ps.release()

    nc.compile()
    return nc


def _get_nc():
    if "nc" not in _CACHE:
        _CACHE["nc"] = _build()
    return _CACHE["nc"]


def _shard_inputs(x_q, x_kv, Wq, bq, Wkv, bkv, Wo=None, bo=None):
    import ml_dtypes

    bf = ml_dtypes.bfloat16
    in_maps = []
    for core in range(NCORES):
        b = core // TPG
        g = core % TPG
        cols = slice(g * CL, (g + 1) * CL)
        m = {
            "xqt": np.ascontiguousarray(x_q[b].T.astype(bf)),
            "xkvt": np.ascontiguousarray(x_kv[b].T.astype(bf)),
            "wq": np.ascontiguousarray(Wq[:, cols].astype(bf)),
            "wk": np.ascontiguousarray(Wkv[:, :C][:, cols].astype(bf)),
            "wv": np.ascontiguousarray(Wkv[:, C:][:, cols].astype(bf)),
            "bq": np.ascontiguousarray(bq[cols].astype(np.float32)),
            "bk": np.ascontiguousarray(bkv[:C][cols].astype(np.float32)),
        }
        if Wo is not None:
            m["wo"] = np.ascontiguousarray(Wo[g * CL : (g + 1) * CL, :].astype(bf))
        in_maps.append(m)
    return in_maps


def kernel(x_q, x_kv, Wq, bq, Wkv, bkv, Wo, bo):
    from concourse.bass_utils import run_bass_kernel_spmd

    x_q = np.asarray(x_q, dtype=np.float32)
    x_kv = np.asarray(x_kv, dtype=np.float32)
    Wq = np.asarray(Wq, dtype=np.float32)
    bq = np.asarray(bq, dtype=np.float32)
    Wkv = np.asarray(Wkv, dtype=np.float32)
    bkv = np.asarray(bkv, dtype=np.float32)
    Wo = np.asarray(Wo, dtype=np.float32)
    bo = np.asarray(bo, dtype=np.float32)

    nc = _get_nc()
    in_maps = _shard_inputs(x_q, x_kv, Wq, bq, Wkv, bkv, Wo, bo)

    res = run_bass_kernel_spmd(nc, in_maps, core_ids=list(range(NCORES)))

    # host-side gather: sum tensor-parallel partials; add exact bias terms
    bias_full = bkv[C:] @ Wo + bo  # v-bias through Wo, plus output bias
    out = np.zeros((B, T, C), dtype=np.float32)
    for core in range(NCORES):
        out[core // TPG] += np.asarray(res.results[core]["out"], dtype=np.float32)
    out += bias_full[None, None, :]
    return out


# revision 17
# speedup vs baseline: 1.0240x; 1.0149x over previous
"""Cross-attention Bass/Tile kernel for Trainium2, sharded over 8 NeuronCores.

Problem (fixed shapes): B=2, T=2048, C=1024, H=16 heads, D=64.
    q = x_q @ Wq + bq;  kv = x_kv @ Wkv + bkv;  k, v = split(kv)
    y = softmax(q k^T / sqrt(D)) v;  out = y @ Wo + bo

Sharding: 8 cores = 2 (batch) x 4 (head groups of 4 heads, 256 channels).
Each core computes its head-group's projections + attention + a partial
output projection; the host sums the 4 partials per batch and adds the
exact v-bias/output-bias terms (att rows sum to 1 => out += bv@Wo + bo).

v2 design (all on-chip compute bf16, PSUM f32):
  - x_q^T / x_kv^T are transposed + bf16-cast on the HOST and passed as
    inputs, eliminating all on-device PE transposes of x.
  - Projections: Q^T,K^T = Wq/k^T x^T (lhsT = W natural), V = x_kv@Wv
    natural (lhsT = x^T chunk).
  - Attention per (qb of 512 q, hc head-pair): S^T[tk,q] (row-packed 2
    heads), exp on ACT ([128,1024] insts, scale=1/8), then AV flipped:
    y[q, d] with lhsT = e2 chunks, rhs = V (free=64: bf16 runs 1
    cycle/row at any free size -> half the cost of the y^T orientation).
    Denominators via free-1 matmuls against a ones vector.
  - Normalization: DVE reciprocal + broadcast multiply; y -> y^T via the
    XBAR dma-transpose engine (14ns/16x128 tile), not PE.
  - Output projection from y^T; PSUM->SBUF bf16 copy on GPSIMD (idle
    engine), DMA out as bf16; host converts/sums in f32.

PSUM map (8 banks): smega [128,2,1024] f32 (S slots, tk-parity) = 4,
ymega [128,2,4,2,64] f32 (y slots, pass-parity) = 2, dps [128,2,8]
(denominators) = 1, utilB [128,512] = 1.  Phase-A projections borrow
smega/ymega as scratch before the first attention pass; woven units
(Q-proj of qb+1, PO of qb-1, phase-A leftovers) rotate through utilB and
the idle ymega slot.  PSUM "start" zeroing is bank-granular, so each
shared bank keeps exactly one open accumulation group window at a time
(start on first slice, stop on last).
"""

import numpy as np

B = 2
T = 2048
C = 1024
H = 16
D = 64
NCORES = 8
TPG = 4  # tensor-parallel group size (head groups)
HL = H // TPG  # heads per core = 4
CL = HL * D  # local channels = 256
P = 128
KC = C // P  # 8 contraction chunks for the projections
NT = T // P  # 16 token chunks of 128
NQ = 4  # q blocks of 512
QW = T // NQ  # 512
DC = CL // P  # 2 chunks of d_local (head pairs)

_CACHE = {}


def _build(no_denom=False, no_weave=False, lag=2, e2_bufs=4, N_WARM=0, N_PREFIX=2):
    import concourse.tile as tile
    from concourse import bacc, mybir

    f32 = mybir.dt.float32
    bf16 = mybir.dt.bfloat16
    Exp = mybir.ActivationFunctionType.Exp

    nc = bacc.Bacc("TRN2", target_bir_lowering=False, debug=False)

    xqt_d = nc.dram_tensor("xqt", [C, T], bf16, kind="ExternalInput")
    xkvt_d = nc.dram_tensor("xkvt", [C, T], bf16, kind="ExternalInput")
    wq_d = nc.dram_tensor("wq", [C, CL], bf16, kind="ExternalInput")
    wk_d = nc.dram_tensor("wk", [C, CL], bf16, kind="ExternalInput")
    wv_d = nc.dram_tensor("wv", [C, CL], bf16, kind="ExternalInput")
    wo_d = nc.dram_tensor("wo", [CL, C], bf16, kind="ExternalInput")
    bq_d = nc.dram_tensor("bq", [CL], f32, kind="ExternalInput")
    bk_d = nc.dram_tensor("bk", [CL], f32, kind="ExternalInput")
    out_d = nc.dram_tensor("out", [T, C], bf16, kind="ExternalOutput")

    with tile.TileContext(nc) as tc:
        with (
            tc.tile_pool(name="const", bufs=1) as const,
            tc.tile_pool(name="persist", bufs=1) as persist,
            tc.tile_pool(name="e2p", bufs=e2_bufs) as e2p,
            tc.tile_pool(name="small", bufs=2) as small,
            tc.tile_pool(name="ostage", bufs=4) as ostage,
        ):
            from concourse.masks import make_identity

            ones_bf = const.tile([P, 1], bf16)
            nc.vector.memset(ones_bf, 1.0)
            ident_f32 = const.tile([P, P], f32)
            make_identity(nc, ident_f32)
            ident_bf = const.tile([P, P], bf16)
            nc.vector.tensor_copy(ident_bf, ident_f32)

            wq_sb = const.tile([P, KC, CL], bf16)
            wk_sb = const.tile([P, KC, CL], bf16)
            wv_sb = const.tile([P, KC, CL], bf16)
            wo_sb = const.tile([P, DC, C], bf16)
            bq_sb = const.tile([P, DC], f32)
            bk_sb = const.tile([P, DC], f32)

            xqt_sb = persist.tile([P, KC, T], bf16)
            xkvt_sb = persist.tile([P, KC, T], bf16)
            qt_sb = persist.tile([P, DC, T], bf16)  # Q^T [d, t]
            kt_sb = persist.tile([P, DC, T], bf16)  # K^T [d, t]
            v_sb = persist.tile([P, NT, HL, D], bf16)  # V [tk, h, d]
            yt_sb = persist.tile([P, DC, T], bf16)  # y^T [d, t] normalized

            ps = tc.alloc_tile_pool(name="ps", bufs=1, space="PSUM")
            smega_t = [
                ps.tile([P, 2 * QW], f32, tag=f"smega{i}", name=f"smega{i}")
                for i in range(2)
            ]
            ymega_t = [
                ps.tile([P, NQ, 2, D], f32, tag=f"ymega{i}", name=f"ymega{i}")
                for i in range(2)
            ]
            dps = ps.tile([P, 2, 8], f32, tag="dps", name="dps")
            dps_t = [dps[:, 0], dps[:, 1]]
            utilB = ps.tile([P, QW], f32, tag="utilB", name="utilB")

            # ---- DMA emission: SWDGE (Pool) carries wk/wv/xkvt, HWDGE
            # (SP) carries wq/xqt/biases/wo.  Order = priority: the
            # phase-A prefix needs wk+xkvt(tb0) and wq+xqt(qb0) first.
            wk_src = wk_d.rearrange("(o p) d -> p o d", p=P)
            wv_src = wv_d.rearrange("(o p) d -> p o d", p=P)
            wq_src = wq_d.rearrange("(o p) d -> p o d", p=P)
            xkvt_src = xkvt_d.rearrange("(o p) t -> p o t", p=P)
            xqt_src = xqt_d.rearrange("(o p) t -> p o t", p=P)

            def xsl(tb):
                return slice(tb * QW, (tb + 1) * QW)

            nc.sync.dma_start(bk_sb, bk_d.rearrange("(o p) -> p o", p=P))
            nc.sync.dma_start(bq_sb, bq_d.rearrange("(o p) -> p o", p=P))
            nc.sync.dma_start(wk_sb, wk_src)
            nc.sync.dma_start(xkvt_sb[:, :, xsl(0)], xkvt_src[:, :, xsl(0)])
            nc.sync.dma_start(wq_sb, wq_src)
            nc.sync.dma_start(xqt_sb[:, :, xsl(0)], xqt_src[:, :, xsl(0)])
            nc.sync.dma_start(wv_sb, wv_src)
            for tb in range(1, NQ):
                nc.sync.dma_start(xkvt_sb[:, :, xsl(tb)], xkvt_src[:, :, xsl(tb)])
            nc.sync.dma_start(xqt_sb[:, :, xsl(1)], xqt_src[:, :, xsl(1)])
            nc.sync.dma_start(wo_sb, wo_d.rearrange("(o p) n -> p o n", p=P))
            nc.sync.dma_start(xqt_sb[:, :, xsl(2)], xqt_src[:, :, xsl(2)])
            nc.sync.dma_start(xqt_sb[:, :, xsl(3)], xqt_src[:, :, xsl(3)])

            # ---------- work units (each takes a [P, QW] f32 PSUM view) ----
            def u_proj(kind, idx, dc):
                x_sb, w_sb, b_sb, dst = (
                    (xqt_sb, wq_sb, bq_sb, qt_sb)
                    if kind == "q"
                    else (xkvt_sb, wk_sb, bk_sb, kt_sb)
                )

                def run(psv):
                    for c in range(KC):
                        nc.tensor.matmul(
                            psv,
                            w_sb[:, c, dc * P : (dc + 1) * P],
                            x_sb[:, c, idx * QW : (idx + 1) * QW],
                            start=(c == 0),
                            stop=(c == KC - 1),
                        )
                    nc.vector.tensor_scalar_add(
                        dst[:, dc, idx * QW : (idx + 1) * QW],
                        psv,
                        b_sb[:, dc : dc + 1],
                    )

                return run, 2.4

            def u_v(tkc):
                def run(psv):
                    v256 = psv[:, 0:CL]
                    for c in range(KC):
                        nc.tensor.matmul(
                            v256,
                            xkvt_sb[:, c, tkc * P : (tkc + 1) * P],
                            wv_sb[:, c, :],
                            start=(c == 0),
                            stop=(c == KC - 1),
                        )
                    nc.vector.tensor_copy(
                        v_sb[:, tkc], v256.rearrange("p (h d) -> p h d", h=HL)
                    )

                return run, 0.9

            def u_po(tch, half, eng=None):
                def run(psv):
                    for dc in range(DC):
                        nc.tensor.matmul(
                            psv,
                            yt_sb[:, dc, tch * P : (tch + 1) * P],
                            wo_sb[:, dc, half * QW : (half + 1) * QW],
                            start=(dc == 0),
                            stop=(dc == DC - 1),
                        )
                    o_st = ostage.tile([P, QW], bf16, tag="o", name="o_st")
                    (eng or nc.gpsimd).tensor_copy(o_st, psv)
                    nc.sync.dma_start(
                        out_d[tch * P : (tch + 1) * P, half * QW : (half + 1) * QW],
                        o_st,
                    )

                return run, 0.6

            # ---- PE warmup: harmless transposes during the DMA lead-in
            # keep the p-state ramp running so phase A runs at full clock
            for i in range(N_WARM):
                wps = utilB[:, 256 + (i % 4) * 64 : 320 + (i % 4) * 64].bitcast(bf16)
                nc.tensor.transpose(wps, ident_bf, ident_bf)

            # ---- phase A prefix: first N_PREFIX units run before pass 0,
            # rotating over the full idle PSUM (deeper pipelining than the
            # in-pass 2-home rotation).
            master = [
                u_proj("k", 0, 0),
                u_proj("q", 0, 0),
                u_v(0), u_v(1), u_v(2), u_v(3),
                u_proj("k", 1, 0),
                u_v(4), u_v(5), u_v(6),
                u_proj("k", 2, 0),
                u_v(7), u_v(8),
                u_proj("k", 3, 0),
                u_v(9), u_v(10), u_v(11),
                u_proj("k", 0, 1),
                u_v(12), u_v(13), u_v(14), u_v(15),
            ]
            prefix_homes = [
                smega_t[0][:, 0:QW],
                smega_t[0][:, QW : 2 * QW],
                smega_t[1][:, 0:QW],
                smega_t[1][:, QW : 2 * QW],
                ymega_t[0].rearrange("p a b d -> p (a b d)"),
                ymega_t[1].rearrange("p a b d -> p (a b d)"),
                utilB,
            ]
            n_pre = max(2, N_PREFIX)
            for i, (run, _cost) in enumerate(master[:n_pre]):
                run(prefix_homes[i % 7])
            rest = master[n_pre:]

            # ---- attention passes ----
            def av_emit(tk, e2, hc, ydst, ddst):
                for qc in range(NQ):
                    for hh in range(2):
                        lhs = e2[:, hh * QW + qc * P : hh * QW + (qc + 1) * P]
                        nc.tensor.matmul(
                            ydst[:, qc, hh],
                            lhs,
                            v_sb[:, tk, hc * 2 + hh],
                            start=(tk == 0 and qc == 0 and hh == 0),
                            stop=(tk == NT - 1 and qc == NQ - 1 and hh == 1),
                        )
                if no_denom:
                    return
                for qc in range(NQ):
                    for hh in range(2):
                        s = qc * 2 + hh
                        lhs = e2[:, hh * QW + qc * P : hh * QW + (qc + 1) * P]
                        nc.tensor.matmul(
                            ddst[:, s : s + 1],
                            lhs,
                            ones_bf,
                            start=(tk == 0 and s == 0),
                            stop=(tk == NT - 1 and s == 7),
                        )

            def emit_pass(qb, hc, pass_idx, weave, prev_tail=None, po_after_qc=None):
                hcpar = pass_idx % 2
                other = 1 - hcpar
                ydst = ymega_t[hcpar]  # [P, NQ, 2, D]
                ddst = dps_t[hcpar]  # [P, 8]
                homes = [
                    utilB,
                    ymega_t[other].rearrange("p a b d -> p (a b d)"),
                ]
                home_i = [0]

                def pop_units(budget):
                    while weave and budget > 0.0:
                        run, cost = weave.pop(0)
                        run(homes[home_i[0] % 2])
                        home_i[0] += 1
                        budget -= cost

                pend = []
                for tk in range(NT):
                    par = tk % 2
                    for hh in range(2):
                        nc.tensor.matmul(
                            smega_t[par][:, hh * QW : (hh + 1) * QW],
                            kt_sb[hh * 64 : (hh + 1) * 64, hc, tk * P : (tk + 1) * P],
                            qt_sb[hh * 64 : (hh + 1) * 64, hc, qb * QW : (qb + 1) * QW],
                            start=True,
                            stop=True,
                            tile_position=(hh * 64, 0),
                        )
                    e2 = e2p.tile([P, 2 * QW], bf16, tag="e2", name="e2")
                    nc.scalar.activation(e2, smega_t[par], Exp, scale=0.125)
                    pend.append((tk, e2))
                    if len(pend) > lag:
                        ptk, pe2 = pend.pop(0)
                        av_emit(ptk, pe2, hc, ydst, ddst)
                    if prev_tail and tk >= 1:
                        prev_tail.pop(0)()
                    else:
                        pop_units(1.6)
                while prev_tail:
                    prev_tail.pop(0)()

                def t_av():
                    while pend:
                        ptk, pe2 = pend.pop(0)
                        av_emit(ptk, pe2, hc, ydst, ddst)
                    while weave:
                        run, _ = weave.pop(0)
                        run(homes[home_i[0] % 2])
                        home_i[0] += 1

                yn = small.tile([P, NQ, 2, D], bf16, tag="yn", name="yn")

                def t_norm():
                    recip = small.tile([P, 8], f32, tag="recip", name="recip")
                    dsrc = (
                        ddst
                        if not no_denom
                        else ymega_t[hcpar].rearrange("p a b d -> p (a b d)")[:, 0:8]
                    )
                    nc.vector.reciprocal(recip, dsrc)
                    try:
                        rb = (
                            recip.rearrange("p (a b) -> p a b", a=NQ)
                            .unsqueeze(-1)
                            .broadcast_to((P, NQ, 2, D))
                        )
                        nc.vector.tensor_mul(out=yn, in0=ydst, in1=rb)
                    except Exception:
                        for s in range(8):
                            qc, hh = s // 2, s % 2
                            nc.vector.tensor_scalar_mul(
                                yn[:, qc, hh], ydst[:, qc, hh], recip[:, s : s + 1]
                            )

                def t_tp(qc):
                    def run():
                        tp_ps = utilB[:, qc * 64 : (qc + 1) * 64].bitcast(bf16)
                        nc.tensor.transpose(
                            tp_ps,
                            yn[:, qc].rearrange("p a d -> p (a d)"),
                            ident_bf,
                        )
                        nc.vector.tensor_copy(
                            yt_sb[:, hc, qb * QW + qc * P : qb * QW + (qc + 1) * P],
                            tp_ps,
                        )
                        if po_after_qc is not None:
                            for prun, home in po_after_qc(qc):
                                prun(home)

                    return run

                return [t_av, t_norm] + [t_tp(qc) for qc in range(NQ)]

            # weave lists per pass (see docstring): phase-A leftovers into
            # pass 0/1, Q-proj of qb+1 into (qb, hc1), PO of qb-1 into
            # (qb, hc0)/(qb, hc1).
            weaves = [[] for _ in range(8)]
            weaves[0] = rest
            weaves[1] = [
                u_proj("k", 1, 1),
                u_proj("q", 1, 0),
                u_proj("k", 2, 1),
                u_proj("q", 1, 1),
                u_proj("k", 3, 1),
            ]
            weaves[2] = [u_po(0 * 4 + i, h) for i in range(4) for h in range(2)]
            weaves[3] = [u_proj("q", 2, 0), u_proj("q", 2, 1)]
            weaves[4] = [u_po(1 * 4 + i, h) for i in range(4) for h in range(2)]
            weaves[5] = [u_proj("q", 3, 0), u_proj("q", 3, 1)]
            weaves[6] = [u_po(2 * 4 + i, h) for i in range(4) for h in range(2)]
            weaves[7] = []

            if no_weave:
                homesA = [
                    smega_t[i // 2][:, (i % 2) * QW : (i % 2 + 1) * QW]
                    for i in range(4)
                ] + [
                    ymega_t[0].rearrange("p a b d -> p (a b d)"),
                    ymega_t[1].rearrange("p a b d -> p (a b d)"),
                    utilB,
                ]
                k = 0
                for w in weaves:
                    while w:
                        run, _ = w.pop(0)
                        run(homesA[k % 7])
                        k += 1
            tail_homes = [
                smega_t[0][:, 0:QW],
                smega_t[0][:, QW : 2 * QW],
                smega_t[1][:, 0:QW],
                smega_t[1][:, QW : 2 * QW],
                ymega_t[0].rearrange("p a b d -> p (a b d)"),
                ymega_t[1].rearrange("p a b d -> p (a b d)"),
                smega_t[0][:, 0:QW],
                smega_t[0][:, QW : 2 * QW],
            ]

            def last_po(qc):
                return [
                    (
                        u_po(3 * 4 + qc, h, eng=(nc.vector if h == 0 else nc.gpsimd))[0],
                        tail_homes[qc * 2 + h],
                    )
                    for h in range(2)
                ]

            prev_tail = None
            pass_idx = 0
            for qb in range(NQ):
                for hc in range(DC):
                    prev_tail = emit_pass(
                        qb,
                        hc,
                        pass_idx,
                        weaves[pass_idx],
                        prev_tail,
                        po_after_qc=(last_po if pass_idx == 7 else None),
                    )
                    pass_idx += 1
            for piece in prev_tail:
                piece()

            ps.release()

    nc.compile()
    return nc


def _get_nc():
    if "nc" not in _CACHE:
        _CACHE["nc"] = _build()
    return _CACHE["nc"]


def _shard_inputs(x_q, x_kv, Wq, bq, Wkv, bkv, Wo=None, bo=None):
    import ml_dtypes

    bf = ml_dtypes.bfloat16
    in_maps = []
    for core in range(NCORES):
        b = core // TPG
        g = core % TPG
        cols = slice(g * CL, (g + 1) * CL)
        m = {
            "xqt": np.ascontiguousarray(x_q[b].T.astype(bf)),
            "xkvt": np.ascontiguousarray(x_kv[b].T.astype(bf)),
            "wq": np.ascontiguousarray(Wq[:, cols].astype(bf)),
            "wk": np.ascontiguousarray(Wkv[:, :C][:, cols].astype(bf)),
            "wv": np.ascontiguousarray(Wkv[:, C:][:, cols].astype(bf)),
            "bq": np.ascontiguousarray(bq[cols].astype(np.float32)),
            "bk": np.ascontiguousarray(bkv[:C][cols].astype(np.float32)),
        }
        if Wo is not None:
            m["wo"] = np.ascontiguousarray(Wo[g * CL : (g + 1) * CL, :].astype(bf))
        in_maps.append(m)
    return in_maps


def kernel(x_q, x_kv, Wq, bq, Wkv, bkv, Wo, bo):
    from concourse.bass_utils import run_bass_kernel_spmd

    x_q = np.asarray(x_q, dtype=np.float32)
    x_kv = np.asarray(x_kv, dtype=np.float32)
    Wq = np.asarray(Wq, dtype=np.float32)
    bq = np.asarray(bq, dtype=np.float32)
    Wkv = np.asarray(Wkv, dtype=np.float32)
    bkv = np.asarray(bkv, dtype=np.float32)
    Wo = np.asarray(Wo, dtype=np.float32)
    bo = np.asarray(bo, dtype=np.float32)

    nc = _get_nc()
    in_maps = _shard_inputs(x_q, x_kv, Wq, bq, Wkv, bkv, Wo, bo)

    res = run_bass_kernel_spmd(nc, in_maps, core_ids=list(range(NCORES)))

    # host-side gather: sum tensor-parallel partials; add exact bias terms
    bias_full = bkv[C:] @ Wo + bo  # v-bias through Wo, plus output bias
    out = np.zeros((B, T, C), dtype=np.float32)
    for core in range(NCORES):
        out[core // TPG] += np.asarray(res.results[core]["out"], dtype=np.float32)
    out += bias_full[None, None, :]
    return out
